# revision 29
# baseline (speedup 1.0000x reference)
"""Trainium2 Bass kernel for nn_Cross_Attention (8-core data-parallel over batch).

Per batch item (one NeuronCore):
  kvf  = conv1x1(kv, qkv1_w)                    # [384, H, W]
  kvd  = depthwise3x3(kvf, qkv2_w, pad=1)       # [384, H, W]
  k, v = split(kvd)
  G    = q_raw @ k_raw^T  (full 192x192 Gram, contracted over pixels)
  attn = softmax(G * scale/|q_i| * 1/|k_j| + blockdiag_mask)
  out  = (Wp @ attn) @ v          # proj folded into attention matrix

Key structure:
 - depthwise as diagonal-weight matmuls with a row-padded kvf layout
   ([130 rows x 130 cols] per 128-ch block) so every tap's rhs is a
   [4,128]-stride-130 AP (256B runs -> full PE stream rate).
 - mc block order (1, 2, 0): k channels 128-191 finish early, k 0-127
   are produced last and Gram accumulation (on raw q/k) is interleaved
   chunk-by-chunk with the last depthwise block. Norms are folded into
   the logits afterwards (row scale = per-partition scalar; column
   scale 1/|k_j| via a K=1 outer-product matmul broadcast).
 - v spilled to DRAM as f16, read back during attn@v.
 - f32<->f16 casts ride inside SWDGE DMAs (gpsimd).
"""

import os
import sys

sys.path.insert(0, "/opt/trn_rl_repo")

NOILV = bool(int(os.environ.get("BASS_CA_NOILV", "0")))
DBG = bool(int(os.environ.get("BASS_CA_DBG", "0")))

import numpy as np

import concourse.bass as bass
import concourse.tile as tile
from concourse import bacc, mybir
from concourse.bass_utils import run_bass_kernel_spmd
from concourse.tile_rust import add_dep_helper
from concourse.bass_interp import get_hw_module

F32 = mybir.dt.float32
F16 = mybir.dt.float16

C = 192          # input channels
C2 = 384         # conv1 output channels
HEADS = 8
CD = C // HEADS
W = 128
H = 128
HWTOT = H * W    # 16384
PT = 512         # pixels per matmul tile
NT = HWTOT // PT  # 32
RPT = PT // W    # 4 rows per tile
LC = 130         # padded kvf cols (1 left pad, 1 right pad)
LR = 130         # padded kvf rows
EPS = 1e-12
MC_ORDER = (1, 2, 0)
NCH = 8          # norm/Gram chunks
CHW = HWTOT // NCH  # 2048 pixels per chunk
TCH = CHW // W   # 16 t-steps per chunk

TAPS = [(dr, dc) for dr in (-1, 0, 1) for dc in (-1, 0, 1)]


def sl(i, size=PT):
    return slice(i * size, (i + 1) * size)


def emit_kernel(tc, io):
    nc = tc.nc
    kv, q = io["kv"], io["q"]
    w1t, w2d, wpt, mask = io["w1t"], io["w2d"], io["wpt"], io["mask"]
    scale192, ident = io["scale192"], io["ident"]
    out, vdram = io["out"], io["vdram"]

    from contextlib import ExitStack
    _stack = ExitStack()
    wp = _stack.enter_context(tc.tile_pool(name="weights", bufs=1))
    big = _stack.enter_context(tc.tile_pool(name="big", bufs=1))
    sml = _stack.enter_context(tc.tile_pool(name="small", bufs=1))

    # ---- weights ----
    w1ta = wp.tile([128, C2], F16); nc.sync.dma_start(w1ta[:], w1t[0:128, :])
    w1tb = wp.tile([64, C2], F16); nc.sync.dma_start(w1tb[:], w1t[128:C, :])
    wpta = wp.tile([128, C], F16); nc.sync.dma_start(wpta[:], wpt[0:128, :])
    wptb = wp.tile([64, C], F16); nc.sync.dma_start(wptb[:], wpt[128:C, :])
    maska = wp.tile([128, C], F32); nc.sync.dma_start(maska[:], mask[0:128, :])
    maskb = wp.tile([64, C], F32); nc.sync.dma_start(maskb[:], mask[128:C, :])
    sca = wp.tile([128, 1], F32); nc.sync.dma_start(sca[:], scale192[0:128, :])
    scb = wp.tile([64, 1], F32); nc.sync.dma_start(scb[:], scale192[128:C, :])
    id16 = wp.tile([128, 128], F16); nc.sync.dma_start(id16[:], ident[:])
    w2sb = wp.tile([128, 27, 128], F16)
    nc.sync.dma_start(w2sb[:], w2d.rearrange("t p c -> p t c"))
    ones1 = wp.tile([1, 128], F16); nc.vector.memset(ones1[:], 1.0)

    # ---- big persistent tiles ----
    kv16 = big.tile([128, 2 * HWTOT], F16, tag="slotA", name="kv16")
    kv16a = kv16[:, 0:HWTOT]
    kv16b = kv16[0:64, HWTOT:2 * HWTOT]
    k16 = big.tile([128, 2 * HWTOT], F16, tag="slotC", name="k16")
    k16a = k16[:, 0:HWTOT]          # k channels 0-127   (mc0)
    k16b = k16[0:64, HWTOT:]        # k channels 128-191 (mc1 lower)

    # small persistent
    qpart = sml.tile([128, NCH], F32)
    qpartb = sml.tile([64, NCH], F32)
    kpart = sml.tile([128, NCH], F32)
    kpartb = sml.tile([64, NCH], F32)
    spa = sml.tile([128, 1], F32)
    spb = sml.tile([64, 1], F32)
    invka = sml.tile([128, 1], F32)
    invkb = sml.tile([64, 1], F32)
    invk16a = sml.tile([128, 1], F16)
    invk16b = sml.tile([64, 1], F16)
    invkrow = sml.tile([1, C], F16)
    at16a = sml.tile([128, C], F16)
    at16b = sml.tile([64, C], F16)
    mt16a = sml.tile([128, C], F16)
    mt16b = sml.tile([64, C], F16)
    ikba = sml.tile([128, C], F32)
    ikbb = sml.tile([64, C], F32)

    # ---- load kv (cast f32 -> f16 in DMA), a/b interleaved ----
    for ch in range(8):
        nc.gpsimd.dma_start(kv16a[:, sl(ch, 2048)], kv[0:128, sl(ch, 2048)])
        nc.gpsimd.dma_start(kv16b[:, sl(ch, 2048)], kv[128:C, sl(ch, 2048)])

    # Gram accumulators live across the whole conv/dw + gram phase
    psG = _stack.enter_context(tc.tile_pool(name="psG", bufs=1, space="PSUM"))
    G0 = psG.tile([128, C], F32, tag="G0", name="G0")
    G1 = psG.tile([64, C], F32, tag="G1", name="G1")

    gram_mm = [0]  # count of emitted Gram accumulation steps (of 2*H)

    # ================= conv1 + depthwise, mc blocks =================
    with tc.tile_pool(name="psC", bufs=1, space="PSUM") as psC, \
         tc.tile_pool(name="psD", bufs=3, space="PSUM") as psD, \
         tc.tile_pool(name="kvfp", bufs=1) as kvfp, \
         tc.tile_pool(name="qstage", bufs=2) as qst, \
         tc.tile_pool(name="vst", bufs=3) as vst:

        CHB = TCH * C  # 3072 f16 elems per chunk buffer
        if DBG:
            dbg_kba_t = sml.tile([128, 4, 128], F16, name="dbgkba")
            dbg_qba_t = sml.tile([128, 4, 128], F16, name="dbgqba")
        k16a_evacs = {}
        spills1 = {}
        spills2 = {}
        qloads = {}
        qtrans = {}
        ktrans = {}

        def emit_q_chunk(c):
            qsa = qst.tile([128, CHW], F16, tag="qsa")
            la = nc.gpsimd.dma_start(qsa[:], q[0:128, sl(c, CHW)])
            qsb = qst.tile([64, CHW], F16, tag="qsb")
            lb = nc.gpsimd.dma_start(qsb[:], q[128:C, sl(c, CHW)])
            qloads[c] = (la, lb)
            if c - 2 in qtrans:
                for t in qtrans[c - 2]:
                    add_dep_helper(la.ins, t.ins, reason="qstage WAR")
                    add_dep_helper(lb.ins, t.ins, reason="qstage WAR")
            qsq = qst.tile([128, CHW], F16, tag="sqt")
            nc.scalar.activation(qsq[:], qsa[:],
                                 mybir.ActivationFunctionType.Square,
                                 accum_out=qpart[:, c:c + 1])
            nc.scalar.activation(qsq[0:64, :], qsb[:],
                                 mybir.ActivationFunctionType.Square,
                                 accum_out=qpartb[:, c:c + 1])
            return qsa, qsb

        def chunk_views(c):
            # contiguous per-buffer sub-tiles: qBa, qBb, kBa, kBb
            b = c % 2
            base = b * 2 * CHB
            qBa = chunks[:, base:base + TCH * 128].rearrange(
                "p (t c) -> p t c", c=128)
            qBb = chunks[:, base + TCH * 128:base + TCH * 192].rearrange(
                "p (t c) -> p t c", c=64)
            kBa = chunks[:, base + TCH * 192:base + TCH * 320].rearrange(
                "p (t c) -> p t c", c=128)
            kBb = chunks[:, base + TCH * 320:base + TCH * 384].rearrange(
                "p (t c) -> p t c", c=64)
            return qBa, qBb, kBa, kBb

        def emit_transposes(c, qsa, qsb):
            # NOTE: DmaTransposeAnt input APs are invisible to the Tile
            # dependency tracker (the *Ant classes do not extend InstDMA),
            # so every input-side dependency is added explicitly here.
            qBa, qBb, kBa, kBb = chunk_views(c)
            t0 = nc.scalar.dma_start_transpose(qBa[:], qsa[:])
            t1 = nc.scalar.dma_start_transpose(qBb[:], qsb[:])
            la, lb = qloads[c]
            add_dep_helper(t0.ins, la.ins, reason="q transpose reads qsa")
            add_dep_helper(t1.ins, lb.ins, reason="q transpose reads qsb")
            t2 = nc.sync.dma_start_transpose(kBa[:], k16a[:, sl(c, CHW)])
            for j in range(4 * c, 4 * c + 4):
                add_dep_helper(t2.ins, k16a_evacs[j].ins,
                               reason="k transpose reads k16a chunk")
            t3 = nc.sync.dma_start_transpose(kBb[:], k16b[:, sl(c, CHW)])
            qtrans[c] = (t0, t1)
            ktrans[c] = (t2, t3)

        def emit_gram_mms(c):
            qBa, qBb, kBa, kBb = chunk_views(c)
            if DBG and c == 0:
                nc.vector.tensor_copy(dbg_kba_t[:], kBa[:, 0:4, :])
                nc.vector.tensor_copy(dbg_qba_t[:], qBa[:, 0:4, :])
            for t in range(TCH):
                # ONE start=True per PSUM bank epoch: the hardware start
                # flag clears has_written for the whole bank, so a second
                # start-carrying matmul into the same bank would wipe the
                # first group's bits and break accumulation. With a single
                # clear, every element group self-initializes via its own
                # first write (has_written=0 -> overwrite).
                s0 = gram_mm[0] == 0
                s1 = gram_mm[0] == H - 1
                nc.tensor.matmul(G0[:, 0:128], qBa[:, t, :], kBa[:, t, :],
                                 start=s0, stop=s1, skip_group_check=True)
                nc.tensor.matmul(G0[:, 128:C], qBa[:, t, :], kBb[:, t, :],
                                 start=False, stop=s1, skip_group_check=True)
                nc.tensor.matmul(G1[:, 0:128], qBb[:, t, :], kBa[:, t, :],
                                 start=s0, stop=s1, skip_group_check=True)
                nc.tensor.matmul(G1[:, 128:C], qBb[:, t, :], kBb[:, t, :],
                                 start=False, stop=s1, skip_group_check=True)
                gram_mm[0] += 1
        for mci, mc in enumerate(MC_ORDER):
            kvf = kvfp.tile([128, LR * LC], F16, tag="kvf", name="kvf")
            kvf3 = kvf[:].rearrange("p (r c) -> p r c", c=LC)
            nc.vector.memset(kvf3[:, 0, :], 0.0)
            nc.vector.memset(kvf3[:, LR - 1, :], 0.0)
            nc.vector.memset(kvf3[:, :, 0:1], 0.0)
            nc.vector.memset(kvf3[:, :, LC - 1:LC], 0.0)
            # conv1: groups of 3 tiles; one weight load per phase per group
            for g0 in range(0, NT, 3):
                gts = range(g0, min(g0 + 3, NT))
                pss = {j: psC.tile([128, PT], F32, tag=f"psC{j - g0}",
                                   name="ps") for j in gts}
                for j in gts:
                    nc.tensor.matmul(pss[j][:], w1ta[:, mc * 128:(mc + 1) * 128],
                                     kv16a[:, sl(j)], start=True, stop=False)
                for j in gts:
                    nc.tensor.matmul(pss[j][:], w1tb[:, mc * 128:(mc + 1) * 128],
                                     kv16b[:, sl(j)], start=False, stop=True)
                for j in gts:
                    dst = kvf3[:, 1 + j * RPT:1 + (j + 1) * RPT, 1:1 + W]
                    nc.any.tensor_copy(dst, pss[j][:])
            # depthwise 3x3: 9 diag matmuls per pixel tile, PSUM accumulate
            def dw_tile(j):
                r0 = j * RPT
                pd = psD.tile([128, PT], F32, tag="psD", name="pd")
                for ti, (dr, dc) in enumerate(TAPS):
                    wi = (dr + 1) * 3 + (dc + 1)
                    rhs = kvf3[:, 1 + r0 + dr:1 + r0 + dr + RPT,
                               1 + dc:1 + dc + W]
                    nc.tensor.matmul(pd[:], w2sb[:, mc * 9 + wi, :], rhs,
                                     start=(ti == 0), stop=(ti == 8))
                # evacuate to destination by mc block
                if mc == 0:
                    ev = nc.vector.tensor_copy(k16a[:, sl(j)], pd[:])
                    k16a_evacs[j] = ev
                elif mc == 1:
                    nc.any.tensor_copy(k16b[:, sl(j)], pd[0:64, :])
                    vs = vst.tile([128, PT], F16, tag="vs", name="vs")
                    nc.any.tensor_copy(vs[64:128, :], pd[64:128, :])
                    spills1[j] = nc.sync.dma_start(vdram[0:64, sl(j)],
                                                   vs[64:128, :])
                else:
                    vs = vst.tile([128, PT], F16, tag="vs", name="vs")
                    nc.any.tensor_copy(vs[:], pd[:])
                    spills2[j] = nc.sync.dma_start(vdram[64:C, sl(j)], vs[:])

            if mc != 0:
                for j in range(NT):
                    dw_tile(j)
            else:
                # interleave: dw chunk c -> transposes c -> gram MMs c-1
                for c in range(NCH):
                    qstaged.append(emit_q_chunk(c))
                    for j in range(4 * c, 4 * c + 4):
                        dw_tile(j)
                    qsa, qsb = qstaged[c]
                    if not NOILV:
                        emit_transposes(c, qsa, qsb)
                    nc.scalar.activation(
                        qst.tile([128, CHW], F16, tag="sqt", name="ksqa")[:],
                        k16a[:, sl(c, CHW)],
                        mybir.ActivationFunctionType.Square,
                        accum_out=kpart[:, c:c + 1])
                    if not NOILV and c >= 1:
                        emit_gram_mms(c - 1)
                if NOILV:
                    for c in range(NCH):
                        emit_transposes(c, *qstaged[c])
                        emit_gram_mms(c)
                else:
                    emit_gram_mms(NCH - 1)
            if mci == 0:
                # after mc1 (first block): k16b complete -> its squares
                for c in range(NCH):
                    nc.scalar.activation(
                        qst.tile([64, CHW], F16, tag="sqt", name="ksq")[:],
                        k16b[:, sl(c, CHW)],
                        mybir.ActivationFunctionType.Square,
                        accum_out=kpartb[:, c:c + 1])
            elif mci == 1:
                chunks = big.tile([128, 4 * CHB], F16, tag="slotA",
                                  name="chunks")
                qstaged = []

    # ---- prefetch all of v into k16's columns (k16 dead after the
    # last kB transpose); overlaps the gram/logits tail ----
    vfull = big.tile([128, 2 * HWTOT], F16, tag="slotC", name="vfull")
    va_all = vfull[:, 0:HWTOT]
    vb_all = vfull[0:64, HWTOT:]
    ktr_all = [t for c in sorted(ktrans) for t in ktrans[c]]
    for ch in range(4):
        ld = nc.sync.dma_start(va_all[:, sl(ch, 4096)],
                               vdram[0:128, sl(ch, 4096)])
        for j in range(8 * ch, 8 * ch + 8):
            add_dep_helper(ld.ins, spills1[j].ins, reason="v prefetch RAW")
            add_dep_helper(ld.ins, spills2[j].ins, reason="v prefetch RAW")
        for t in ktr_all:
            add_dep_helper(ld.ins, t.ins, reason="vfull WAR vs k16 transpose")
    for ch in range(4):
        ld = nc.sync.dma_start(vb_all[:, sl(ch, 4096)],
                               vdram[128:C, sl(ch, 4096)])
        for j in range(8 * ch, 8 * ch + 8):
            add_dep_helper(ld.ins, spills2[j].ins, reason="v prefetch RAW")
        for t in ktr_all:
            add_dep_helper(ld.ins, t.ins, reason="vfull WAR vs k16 transpose")

    # ================= norms -> sp, invk =================
    nq2a = sml.tile([128, 1], F32)
    nq2b = sml.tile([64, 1], F32)
    nc.vector.reduce_sum(nq2a[:], qpart[:], axis=mybir.AxisListType.X)
    nc.vector.reduce_sum(nq2b[:], qpartb[:], axis=mybir.AxisListType.X)
    nc.vector.reduce_sum(invka[:], kpart[:], axis=mybir.AxisListType.X)
    nc.vector.reduce_sum(invkb[:], kpartb[:], axis=mybir.AxisListType.X)
    for nrm, scx, dst in ((nq2a, sca, spa), (nq2b, scb, spb)):
        nc.scalar.sqrt(nrm[:], nrm[:])
        nc.vector.tensor_scalar_max(nrm[:], nrm[:], EPS)
        nc.vector.reciprocal(nrm[:], nrm[:])
        nc.vector.tensor_tensor(out=dst[:], in0=nrm[:], in1=scx[:],
                                op=mybir.AluOpType.mult)
    for nrm in (invka, invkb):
        nc.scalar.sqrt(nrm[:], nrm[:])
        nc.vector.tensor_scalar_max(nrm[:], nrm[:], EPS)
        nc.vector.reciprocal(nrm[:], nrm[:])
    # broadcast 1/|k_j| along partitions: transpose to a row, outer-product
    nc.vector.tensor_copy(invk16a[:], invka[:])
    nc.vector.tensor_copy(invk16b[:], invkb[:])
    with tc.tile_pool(name="psB", bufs=1, space="PSUM") as psB:
        rowp = psB.tile([1, 128], F16, tag="rp", name="rowp")
        nc.tensor.transpose(rowp[:], invk16a[:], id16[:])
        nc.vector.tensor_copy(invkrow[0:1, 0:128], rowp[:])
        rowp2 = psB.tile([1, 128], F16, tag="rp2", name="rowp2")
        nc.tensor.transpose(rowp2[0:1, 0:64], invk16b[:],
                            id16[0:64, 0:64])
        nc.vector.tensor_copy(invkrow[0:1, 128:C], rowp2[0:1, 0:64])
        ikA = psB.tile([128, C], F32, tag="ikA", name="ikA")
        nc.tensor.matmul(ikA[:], ones1[0:1, 0:128], invkrow[:],
                         start=True, stop=True)
        ikB = psB.tile([64, C], F32, tag="ikB", name="ikB")
        nc.tensor.matmul(ikB[:], ones1[0:1, 0:64], invkrow[:],
                         start=True, stop=True)
        nc.vector.tensor_copy(ikba[:], ikA[:])
        nc.vector.tensor_copy(ikbb[:], ikB[:])

    # ================= logits, softmax, M^T = A^T Wp^T =================
    with tc.tile_pool(name="smax", bufs=1) as sm, \
         tc.tile_pool(name="psM", bufs=1, space="PSUM") as psM:
        for Gx, ikx, spx, mkx, atx, rows in (
                (G0, ikba, spa, maska, at16a, 128),
                (G1, ikbb, spb, maskb, at16b, 64)):
            lg = sm.tile([rows, C], F32, tag=f"lg{rows}", name=f"lg{rows}")
            nc.vector.tensor_tensor(out=lg[:], in0=Gx[:], in1=ikx[:],
                                    op=mybir.AluOpType.mult)
            lg2 = sm.tile([rows, C], F32, tag=f"lh{rows}", name=f"lh{rows}")
            nc.vector.scalar_tensor_tensor(
                out=lg2[:], in0=lg[:], scalar=spx[:], in1=mkx[:],
                op0=mybir.AluOpType.mult, op1=mybir.AluOpType.add)
            mx = sm.tile([rows, 1], F32, tag=f"mx{rows}", name=f"mx{rows}")
            nc.vector.reduce_max(mx[:], lg2[:], axis=mybir.AxisListType.X)
            nc.vector.tensor_scalar_mul(mx[:], mx[:], -1.0)
            ssum = sm.tile([rows, 1], F32, tag=f"ss{rows}", name=f"ss{rows}")
            nc.scalar.activation(lg2[:], lg2[:],
                                 mybir.ActivationFunctionType.Exp,
                                 bias=mx[:], accum_out=ssum[:])
            nc.vector.reciprocal(ssum[:], ssum[:])
            nc.vector.tensor_scalar_mul(atx[:], lg2[:], ssum[:])
        # MT = A^T @ Wp^T  ([d, o], d on partitions)
        mta = psM.tile([128, C], F32, tag="mta", name="mta")
        nc.tensor.matmul(mta[:], at16a[:, 0:128], wpta[:],
                         start=True, stop=False)
        nc.tensor.matmul(mta[:], at16b[:, 0:128], wptb[:],
                         start=False, stop=True)
        mtb = psM.tile([64, C], F32, tag="mtb", name="mtb")
        nc.tensor.matmul(mtb[:], at16a[:, 128:C], wpta[:],
                         start=True, stop=False)
        nc.tensor.matmul(mtb[:], at16b[:, 128:C], wptb[:],
                         start=False, stop=True)
        nc.scalar.copy(mt16a[:], mta[:])
        nc.scalar.copy(mt16b[:], mtb[:])

    if DBG:
        gsb = sml.tile([128, C], F32)
        gsb2 = sml.tile([64, C], F32)
        nc.vector.tensor_copy(gsb[:], G0[:])
        nc.vector.tensor_copy(gsb2[:], G1[:])
        nc.sync.dma_start(io["dbg_ga"][0:128, :], gsb[:])
        nc.sync.dma_start(io["dbg_gb"][0:64, :], gsb2[:])
        nc.sync.dma_start(io["dbg_at"][0:128, :], at16a[:])
        nc.sync.dma_start(io["dbg_at"][128:C, :], at16b[:])
        nc.sync.dma_start(io["dbg_mt"][0:128, :], mt16a[:])
        nc.sync.dma_start(io["dbg_mt"][128:C, :], mt16b[:])
        nc.sync.dma_start(io["dbg_sp"][0:128, :], spa[:])
        nc.sync.dma_start(io["dbg_sp"][128:C, :], spb[:])
        nc.sync.dma_start(io["dbg_ik"][0:128, :], invka[:])
        nc.sync.dma_start(io["dbg_ik"][128:C, :], invkb[:])
        nc.sync.dma_start(io["dbg_k16"][0:128, :], k16a[:])
        nc.sync.dma_start(io["dbg_k16"][128:C, :], k16b[:])
        nc.sync.dma_start(io["dbg_v16"][0:128, :], va_all[:])
        nc.sync.dma_start(io["dbg_v16"][128:C, :], vb_all[:])
        nc.sync.dma_start(io["dbg_kba"][:, :], dbg_kba_t[:])
        nc.sync.dma_start(io["dbg_qba"][:, :], dbg_qba_t[:])

    # ================= out = M @ v =================
    with tc.tile_pool(name="ost", bufs=2) as ost, \
         tc.tile_pool(name="psO", bufs=1, space="PSUM") as psO:
        oa = ob = None
        for g0 in range(0, NT, 3):
            gts = list(range(g0, min(g0 + 3, NT)))
            O0s = {j: psO.tile([128, PT], F32, tag=f"O0{j - g0}", name="O0")
                   for j in gts}
            O1s = {j: psO.tile([64, PT], F32, tag=f"O1{j - g0}", name="O1")
                   for j in gts}
            for j in gts:
                nc.tensor.matmul(O0s[j][:], mt16a[:, 0:128], va_all[:, sl(j)],
                                 start=True, stop=False)
            for j in gts:
                nc.tensor.matmul(O0s[j][:], mt16b[:, 0:128], vb_all[:, sl(j)],
                                 start=False, stop=True)
            for j in gts:
                nc.tensor.matmul(O1s[j][:], mt16a[:, 128:C], va_all[:, sl(j)],
                                 start=True, stop=False)
            for j in gts:
                nc.tensor.matmul(O1s[j][:], mt16b[:, 128:C], vb_all[:, sl(j)],
                                 start=False, stop=True)
            for j in gts:
                c, jj = j // 4, j % 4
                if jj == 0:
                    oa = ost.tile([128, CHW], F32, tag="oa", name="oa")
                    ob = ost.tile([64, CHW], F32, tag="ob", name="ob")
                    oab = {}
                oab[c] = (oa, ob)
                nc.any.tensor_copy(oa[:, sl(jj)], O0s[j][:])
                nc.any.tensor_copy(ob[:, sl(jj)], O1s[j][:])
                if jj == 3:
                    nc.sync.dma_start(out[0:128, sl(c, CHW)], oa[:])
                    nc.sync.dma_start(out[128:C, sl(c, CHW)], ob[:])
    _stack.close()


def build_module():
    nc = bacc.Bacc("TRN2")
    io = {}
    io["kv"] = nc.dram_tensor("kv", [C, HWTOT], F32, kind="ExternalInput").ap()
    io["q"] = nc.dram_tensor("q", [C, HWTOT], F32, kind="ExternalInput").ap()
    io["w1t"] = nc.dram_tensor("w1t", [C, C2], F16, kind="ExternalInput").ap()
    io["w2d"] = nc.dram_tensor("w2d", [27, 128, 128], F16,
                               kind="ExternalInput").ap()
    io["wpt"] = nc.dram_tensor("wpt", [C, C], F16, kind="ExternalInput").ap()
    io["ident"] = nc.dram_tensor("ident", [128, 128], F16,
                                 kind="ExternalInput").ap()
    io["mask"] = nc.dram_tensor("mask", [C, C], F32, kind="ExternalInput").ap()
    io["scale192"] = nc.dram_tensor("scale192", [C, 1], F32,
                                    kind="ExternalInput").ap()
    io["out"] = nc.dram_tensor("out", [C, HWTOT], F32, kind="ExternalOutput").ap()
    io["vdram"] = nc.dram_tensor("vdram", [C, HWTOT], F16).ap()
    if DBG:
        io["dbg_ga"] = nc.dram_tensor("dbg_ga", [128, C], F32, kind="ExternalOutput").ap()
        io["dbg_gb"] = nc.dram_tensor("dbg_gb", [64, C], F32, kind="ExternalOutput").ap()
        io["dbg_at"] = nc.dram_tensor("dbg_at", [C, C], F16, kind="ExternalOutput").ap()
        io["dbg_mt"] = nc.dram_tensor("dbg_mt", [C, C], F16, kind="ExternalOutput").ap()
        io["dbg_sp"] = nc.dram_tensor("dbg_sp", [C, 1], F32, kind="ExternalOutput").ap()
        io["dbg_ik"] = nc.dram_tensor("dbg_ik", [C, 1], F32, kind="ExternalOutput").ap()
        io["dbg_k16"] = nc.dram_tensor("dbg_k16", [C, HWTOT], F16, kind="ExternalOutput").ap()
        io["dbg_v16"] = nc.dram_tensor("dbg_v16", [C, HWTOT], F16, kind="ExternalOutput").ap()
        io["dbg_kba"] = nc.dram_tensor("dbg_kba", [128, 4 * 128], F16, kind="ExternalOutput").ap()
        io["dbg_qba"] = nc.dram_tensor("dbg_qba", [128, 4 * 128], F16, kind="ExternalOutput").ap()
    with tile.TileContext(nc) as tc:
        emit_kernel(tc, io)
    nc.compile()
    return nc


def prep_weights(qkv1_w, qkv2_w, proj_w, scale):
    w1 = np.asarray(qkv1_w).reshape(C2, C)
    w1t = np.ascontiguousarray(w1.T).astype(np.float16)
    w2 = np.asarray(qkv2_w).reshape(C2, 9)
    w2d = np.zeros((27, 128, 128), np.float16)
    for mc in range(3):
        for wi in range(9):
            np.fill_diagonal(w2d[mc * 9 + wi], w2[mc * 128:(mc + 1) * 128, wi])
    wpr = np.asarray(proj_w).reshape(C, C)
    wpt = np.ascontiguousarray(wpr.T).astype(np.float16)
    ident = np.eye(128, dtype=np.float16)
    mask = np.full((C, C), -1e30, np.float32)
    for h in range(HEADS):
        mask[h * CD:(h + 1) * CD, h * CD:(h + 1) * CD] = 0.0
    scale192 = np.repeat(np.asarray(scale).reshape(HEADS), CD).astype(
        np.float32).reshape(C, 1)
    return {"w1t": w1t, "w2d": w2d, "wpt": wpt, "ident": ident,
            "mask": mask, "scale192": scale192}


_CACHED = {}


def kernel(kv, q, qkv1_w, qkv2_w, proj_w, scale):
    kv = np.asarray(kv, np.float32)
    q = np.asarray(q, np.float32)
    b = kv.shape[0]
    assert b == 8 and kv.shape[1] == C
    wts = prep_weights(qkv1_w, qkv2_w, proj_w, scale)
    if "nc" not in _CACHED:
        ncm = build_module()
        ncm.m = get_hw_module(ncm.m)
        _CACHED["nc"] = ncm
    ncm = _CACHED["nc"]
    in_maps = []
    for i in range(b):
        m = {"kv": np.ascontiguousarray(kv[i].reshape(C, HWTOT)),
             "q": np.ascontiguousarray(q[i].reshape(C, HWTOT))}
        m.update(wts)
        in_maps.append(m)
    res = run_bass_kernel_spmd(ncm, in_maps, core_ids=list(range(8)))
    outv = np.stack([res.results[i]["out"].reshape(C, H, W) for i in range(b)])
    return outv.astype(np.float32)


# revision 30
# speedup vs baseline: 1.0904x; 1.0904x over previous
"""Trainium2 Bass kernel for nn_Cross_Attention (8-core data-parallel over batch).

Per batch item (one NeuronCore):
  kvf  = conv1x1(kv, qkv1_w)                    # [384, H, W]
  kvd  = depthwise3x3(kvf, qkv2_w, pad=1)       # [384, H, W]
  k, v = split(kvd)
  G    = q_raw @ k_raw^T  (full 192x192 Gram, contracted over pixels)
  attn = softmax(G * scale/|q_i| * 1/|k_j| + blockdiag_mask)
  out  = (Wp @ attn) @ v          # proj folded into attention matrix

Key structure:
 - depthwise as diagonal-weight matmuls with a row-padded kvf layout
   ([130 rows x 130 cols] per 128-ch block) so every tap's rhs is a
   [4,128]-stride-130 AP (256B runs -> full PE stream rate).
 - mc block order (1, 2, 0): k channels 128-191 finish early, k 0-127
   are produced last and Gram accumulation (on raw q/k) is interleaved
   chunk-by-chunk with the last depthwise block. Norms are folded into
   the logits afterwards (row scale = per-partition scalar; column
   scale 1/|k_j| via a K=1 outer-product matmul broadcast).
 - v spilled to DRAM as f16, read back during attn@v.
 - f32<->f16 casts ride inside SWDGE DMAs (gpsimd).
"""

import os
import sys

sys.path.insert(0, "/opt/trn_rl_repo")

NOILV = bool(int(os.environ.get("BASS_CA_NOILV", "0")))
DBG = bool(int(os.environ.get("BASS_CA_DBG", "0")))

import numpy as np

import concourse.bass as bass
import concourse.tile as tile
from concourse import bacc, mybir
from concourse.bass_utils import run_bass_kernel_spmd
from concourse.tile_rust import add_dep_helper
from concourse.bass_interp import get_hw_module

F32 = mybir.dt.float32
F16 = mybir.dt.float16

C = 192          # input channels
C2 = 384         # conv1 output channels
HEADS = 8
CD = C // HEADS
W = 128
H = 128
HWTOT = H * W    # 16384
PT = 512         # pixels per matmul tile
NT = HWTOT // PT  # 32
RPT = PT // W    # 4 rows per tile
LC = 130         # padded kvf cols (1 left pad, 1 right pad)
LR = 130         # padded kvf rows
EPS = 1e-12
MC_ORDER = (1, 2, 0)
NCH = 8          # norm/Gram chunks
CHW = HWTOT // NCH  # 2048 pixels per chunk
TCH = CHW // W   # 16 t-steps per chunk

TAPS = [(dr, dc) for dr in (-1, 0, 1) for dc in (-1, 0, 1)]


def sl(i, size=PT):
    return slice(i * size, (i + 1) * size)


def emit_kernel(tc, io):
    nc = tc.nc
    kv, q = io["kv"], io["q"]
    w1t, w2d, wpt, mask = io["w1t"], io["w2d"], io["wpt"], io["mask"]
    scale192, ident = io["scale192"], io["ident"]
    out, vdram = io["out"], io["vdram"]

    from contextlib import ExitStack
    _stack = ExitStack()
    wp = _stack.enter_context(tc.tile_pool(name="weights", bufs=1))
    big = _stack.enter_context(tc.tile_pool(name="big", bufs=1))
    sml = _stack.enter_context(tc.tile_pool(name="small", bufs=1))

    # ---- weights ----
    w1ta = wp.tile([128, C2], F16); nc.sync.dma_start(w1ta[:], w1t[0:128, :])
    w1tb = wp.tile([64, C2], F16); nc.sync.dma_start(w1tb[:], w1t[128:C, :])
    wpta = wp.tile([128, C], F16); nc.sync.dma_start(wpta[:], wpt[0:128, :])
    wptb = wp.tile([64, C], F16); nc.sync.dma_start(wptb[:], wpt[128:C, :])
    maska = wp.tile([128, C], F32); nc.sync.dma_start(maska[:], mask[0:128, :])
    maskb = wp.tile([64, C], F32); nc.sync.dma_start(maskb[:], mask[128:C, :])
    sca = wp.tile([128, 1], F32); nc.sync.dma_start(sca[:], scale192[0:128, :])
    scb = wp.tile([64, 1], F32); nc.sync.dma_start(scb[:], scale192[128:C, :])
    id16 = wp.tile([128, 128], F16); nc.sync.dma_start(id16[:], ident[:])
    w2sb = wp.tile([128, 27, 128], F16)
    nc.sync.dma_start(w2sb[:], w2d.rearrange("t p c -> p t c"))
    ones1 = wp.tile([1, 128], F16); nc.vector.memset(ones1[:], 1.0)

    # ---- big persistent tiles ----
    kv16 = big.tile([128, 2 * HWTOT], F16, tag="slotA", name="kv16")
    kv16a = kv16[:, 0:HWTOT]
    kv16b = kv16[0:64, HWTOT:2 * HWTOT]
    k16 = big.tile([128, 2 * HWTOT], F16, tag="slotC", name="k16")
    k16a = k16[:, 0:HWTOT]          # k channels 0-127   (mc0)
    k16b = k16[0:64, HWTOT:]        # k channels 128-191 (mc1 lower)

    # small persistent
    qpart = sml.tile([128, NCH], F32)
    qpartb = sml.tile([64, NCH], F32)
    kpart = sml.tile([128, NCH], F32)
    kpartb = sml.tile([64, NCH], F32)
    spa = sml.tile([128, 1], F32)
    spb = sml.tile([64, 1], F32)
    invka = sml.tile([128, 1], F32)
    invkb = sml.tile([64, 1], F32)
    invk16a = sml.tile([128, 1], F16)
    invk16b = sml.tile([64, 1], F16)
    invkrow = sml.tile([1, C], F16)
    at16a = sml.tile([128, C], F16)
    at16b = sml.tile([64, C], F16)
    mt16a = sml.tile([128, C], F16)
    mt16b = sml.tile([64, C], F16)
    ikba = sml.tile([128, C], F32)
    ikbb = sml.tile([64, C], F32)

    # ---- load kv (cast f32 -> f16 in DMA), a/b interleaved ----
    for ch in range(8):
        nc.gpsimd.dma_start(kv16a[:, sl(ch, 2048)], kv[0:128, sl(ch, 2048)])
        nc.gpsimd.dma_start(kv16b[:, sl(ch, 2048)], kv[128:C, sl(ch, 2048)])

    # Gram accumulators live across the whole conv/dw + gram phase
    psG = _stack.enter_context(tc.tile_pool(name="psG", bufs=1, space="PSUM"))
    G0 = psG.tile([128, C], F32, tag="G0", name="G0")
    G1 = psG.tile([64, C], F32, tag="G1", name="G1")

    gram_mm = [0]  # count of emitted Gram accumulation steps (of 2*H)

    # ================= conv1 + depthwise, mc blocks =================
    with tc.tile_pool(name="psC", bufs=1, space="PSUM") as psC, \
         tc.tile_pool(name="psD", bufs=3, space="PSUM") as psD, \
         tc.tile_pool(name="kvfp", bufs=1) as kvfp, \
         tc.tile_pool(name="qstage", bufs=2) as qst, \
         tc.tile_pool(name="vst", bufs=3) as vst:

        CHB = TCH * C  # 3072 f16 elems per chunk buffer
        if DBG:
            dbg_kba_t = sml.tile([128, 4, 128], F16, name="dbgkba")
            dbg_qba_t = sml.tile([128, 4, 128], F16, name="dbgqba")
        k16a_evacs = {}
        spills1 = {}
        spills2 = {}
        qloads = {}
        qtrans = {}
        ktrans = {}

        def emit_q_chunk(c):
            qsa = qst.tile([128, CHW], F16, tag="qsa")
            la = nc.gpsimd.dma_start(qsa[:], q[0:128, sl(c, CHW)])
            qsb = qst.tile([64, CHW], F16, tag="qsb")
            lb = nc.gpsimd.dma_start(qsb[:], q[128:C, sl(c, CHW)])
            qloads[c] = (la, lb)
            if c - 2 in qtrans:
                for t in qtrans[c - 2]:
                    add_dep_helper(la.ins, t.ins, reason="qstage WAR")
                    add_dep_helper(lb.ins, t.ins, reason="qstage WAR")
            qsq = qst.tile([128, CHW], F16, tag="sqt")
            nc.scalar.activation(qsq[:], qsa[:],
                                 mybir.ActivationFunctionType.Square,
                                 accum_out=qpart[:, c:c + 1])
            nc.scalar.activation(qsq[0:64, :], qsb[:],
                                 mybir.ActivationFunctionType.Square,
                                 accum_out=qpartb[:, c:c + 1])
            return qsa, qsb

        def chunk_views(c):
            # contiguous per-buffer sub-tiles: qBa, qBb, kBa, kBb
            b = c % 2
            base = b * 2 * CHB
            qBa = chunks[:, base:base + TCH * 128].rearrange(
                "p (t c) -> p t c", c=128)
            qBb = chunks[:, base + TCH * 128:base + TCH * 192].rearrange(
                "p (t c) -> p t c", c=64)
            kBa = chunks[:, base + TCH * 192:base + TCH * 320].rearrange(
                "p (t c) -> p t c", c=128)
            kBb = chunks[:, base + TCH * 320:base + TCH * 384].rearrange(
                "p (t c) -> p t c", c=64)
            return qBa, qBb, kBa, kBb

        def emit_transposes(c, qsa, qsb):
            # NOTE: DmaTransposeAnt input APs are invisible to the Tile
            # dependency tracker (the *Ant classes do not extend InstDMA),
            # so every input-side dependency is added explicitly here.
            qBa, qBb, kBa, kBb = chunk_views(c)
            # NOTE: concurrent DMA transposes on the two HWDGE rings
            # (sync+scalar) corrupt data on HW - keep ALL transposes on sync.
            t0 = nc.sync.dma_start_transpose(qBa[:], qsa[:])
            t1 = nc.sync.dma_start_transpose(qBb[:], qsb[:])
            la, lb = qloads[c]
            add_dep_helper(t0.ins, la.ins, reason="q transpose reads qsa")
            add_dep_helper(t1.ins, lb.ins, reason="q transpose reads qsb")
            t2 = nc.sync.dma_start_transpose(kBa[:], k16a[:, sl(c, CHW)])
            for j in range(4 * c, 4 * c + 4):
                add_dep_helper(t2.ins, k16a_evacs[j].ins,
                               reason="k transpose reads k16a chunk")
            t3 = nc.sync.dma_start_transpose(kBb[:], k16b[:, sl(c, CHW)])
            qtrans[c] = (t0, t1)
            ktrans[c] = (t2, t3)

        def emit_gram_mms(c):
            qBa, qBb, kBa, kBb = chunk_views(c)
            if DBG and c == 0:
                nc.vector.tensor_copy(dbg_kba_t[:], kBa[:, 0:4, :])
                nc.vector.tensor_copy(dbg_qba_t[:], qBa[:, 0:4, :])
            for t in range(TCH):
                # ONE start=True per PSUM bank epoch: the hardware start
                # flag clears has_written for the whole bank, so a second
                # start-carrying matmul into the same bank would wipe the
                # first group's bits and break accumulation. With a single
                # clear, every element group self-initializes via its own
                # first write (has_written=0 -> overwrite).
                s0 = gram_mm[0] == 0
                s1 = gram_mm[0] == H - 1
                nc.tensor.matmul(G0[:, 0:128], qBa[:, t, :], kBa[:, t, :],
                                 start=s0, stop=s1, skip_group_check=True)
                nc.tensor.matmul(G0[:, 128:C], qBa[:, t, :], kBb[:, t, :],
                                 start=False, stop=s1, skip_group_check=True)
                nc.tensor.matmul(G1[:, 0:128], qBb[:, t, :], kBa[:, t, :],
                                 start=s0, stop=s1, skip_group_check=True)
                nc.tensor.matmul(G1[:, 128:C], qBb[:, t, :], kBb[:, t, :],
                                 start=False, stop=s1, skip_group_check=True)
                gram_mm[0] += 1
        for mci, mc in enumerate(MC_ORDER):
            kvf = kvfp.tile([128, LR * LC], F16, tag="kvf", name="kvf")
            kvf3 = kvf[:].rearrange("p (r c) -> p r c", c=LC)
            nc.vector.memset(kvf3[:, 0, :], 0.0)
            nc.vector.memset(kvf3[:, LR - 1, :], 0.0)
            nc.vector.memset(kvf3[:, :, 0:1], 0.0)
            nc.vector.memset(kvf3[:, :, LC - 1:LC], 0.0)
            # conv1: groups of 3 tiles; one weight load per phase per group
            for g0 in range(0, NT, 3):
                gts = range(g0, min(g0 + 3, NT))
                pss = {j: psC.tile([128, PT], F32, tag=f"psC{j - g0}",
                                   name="ps") for j in gts}
                for j in gts:
                    nc.tensor.matmul(pss[j][:], w1ta[:, mc * 128:(mc + 1) * 128],
                                     kv16a[:, sl(j)], start=True, stop=False)
                for j in gts:
                    nc.tensor.matmul(pss[j][:], w1tb[:, mc * 128:(mc + 1) * 128],
                                     kv16b[:, sl(j)], start=False, stop=True)
                for j in gts:
                    dst = kvf3[:, 1 + j * RPT:1 + (j + 1) * RPT, 1:1 + W]
                    nc.any.tensor_copy(dst, pss[j][:])
            # depthwise 3x3: 9 diag matmuls per pixel tile, PSUM accumulate
            def dw_tile(j):
                r0 = j * RPT
                pd = psD.tile([128, PT], F32, tag="psD", name="pd")
                for ti, (dr, dc) in enumerate(TAPS):
                    wi = (dr + 1) * 3 + (dc + 1)
                    rhs = kvf3[:, 1 + r0 + dr:1 + r0 + dr + RPT,
                               1 + dc:1 + dc + W]
                    nc.tensor.matmul(pd[:], w2sb[:, mc * 9 + wi, :], rhs,
                                     start=(ti == 0), stop=(ti == 8))
                # evacuate to destination by mc block
                if mc == 0:
                    ev = nc.vector.tensor_copy(k16a[:, sl(j)], pd[:])
                    k16a_evacs[j] = ev
                elif mc == 1:
                    nc.any.tensor_copy(k16b[:, sl(j)], pd[0:64, :])
                    vs = vst.tile([128, PT], F16, tag="vs", name="vs")
                    nc.any.tensor_copy(vs[64:128, :], pd[64:128, :])
                    spills1[j] = nc.sync.dma_start(vdram[0:64, sl(j)],
                                                   vs[64:128, :])
                else:
                    vs = vst.tile([128, PT], F16, tag="vs", name="vs")
                    nc.any.tensor_copy(vs[:], pd[:])
                    spills2[j] = nc.sync.dma_start(vdram[64:C, sl(j)], vs[:])

            if mc != 0:
                for j in range(NT):
                    dw_tile(j)
            else:
                # interleave: dw chunk c -> transposes c -> gram MMs c-1
                for c in range(NCH):
                    qstaged.append(emit_q_chunk(c))
                    for j in range(4 * c, 4 * c + 4):
                        dw_tile(j)
                    qsa, qsb = qstaged[c]
                    if not NOILV:
                        emit_transposes(c, qsa, qsb)
                    nc.scalar.activation(
                        qst.tile([128, CHW], F16, tag="sqt", name="ksqa")[:],
                        k16a[:, sl(c, CHW)],
                        mybir.ActivationFunctionType.Square,
                        accum_out=kpart[:, c:c + 1])
                    if not NOILV and c >= 1:
                        emit_gram_mms(c - 1)
                if NOILV:
                    for c in range(NCH):
                        emit_transposes(c, *qstaged[c])
                        emit_gram_mms(c)
                else:
                    emit_gram_mms(NCH - 1)
            if mci == 0:
                # after mc1 (first block): k16b complete -> its squares
                for c in range(NCH):
                    nc.scalar.activation(
                        qst.tile([64, CHW], F16, tag="sqt", name="ksq")[:],
                        k16b[:, sl(c, CHW)],
                        mybir.ActivationFunctionType.Square,
                        accum_out=kpartb[:, c:c + 1])
            elif mci == 1:
                chunks = big.tile([128, 4 * CHB], F16, tag="slotA",
                                  name="chunks")
                qstaged = []

    # ---- prefetch all of v into k16's columns (k16 dead after the
    # last kB transpose); overlaps the gram/logits tail ----
    vfull = big.tile([128, 2 * HWTOT], F16, tag="slotC", name="vfull")
    va_all = vfull[:, 0:HWTOT]
    vb_all = vfull[0:64, HWTOT:]
    ktr_all = [t for c in sorted(ktrans) for t in ktrans[c]]
    for ch in range(4):
        ld = nc.sync.dma_start(va_all[:, sl(ch, 4096)],
                               vdram[0:128, sl(ch, 4096)])
        for j in range(8 * ch, 8 * ch + 8):
            add_dep_helper(ld.ins, spills1[j].ins, reason="v prefetch RAW")
            add_dep_helper(ld.ins, spills2[j].ins, reason="v prefetch RAW")
        for t in ktr_all:
            add_dep_helper(ld.ins, t.ins, reason="vfull WAR vs k16 transpose")
    for ch in range(4):
        ld = nc.sync.dma_start(vb_all[:, sl(ch, 4096)],
                               vdram[128:C, sl(ch, 4096)])
        for j in range(8 * ch, 8 * ch + 8):
            add_dep_helper(ld.ins, spills2[j].ins, reason="v prefetch RAW")
        for t in ktr_all:
            add_dep_helper(ld.ins, t.ins, reason="vfull WAR vs k16 transpose")

    # ================= norms -> sp, invk =================
    nq2a = sml.tile([128, 1], F32)
    nq2b = sml.tile([64, 1], F32)
    nc.vector.reduce_sum(nq2a[:], qpart[:], axis=mybir.AxisListType.X)
    nc.vector.reduce_sum(nq2b[:], qpartb[:], axis=mybir.AxisListType.X)
    nc.vector.reduce_sum(invka[:], kpart[:], axis=mybir.AxisListType.X)
    nc.vector.reduce_sum(invkb[:], kpartb[:], axis=mybir.AxisListType.X)
    for nrm, scx, dst in ((nq2a, sca, spa), (nq2b, scb, spb)):
        nc.scalar.sqrt(nrm[:], nrm[:])
        nc.vector.tensor_scalar_max(nrm[:], nrm[:], EPS)
        nc.vector.reciprocal(nrm[:], nrm[:])
        nc.vector.tensor_tensor(out=dst[:], in0=nrm[:], in1=scx[:],
                                op=mybir.AluOpType.mult)
    for nrm in (invka, invkb):
        nc.scalar.sqrt(nrm[:], nrm[:])
        nc.vector.tensor_scalar_max(nrm[:], nrm[:], EPS)
        nc.vector.reciprocal(nrm[:], nrm[:])
    # broadcast 1/|k_j| along partitions: transpose to a row, outer-product
    nc.vector.tensor_copy(invk16a[:], invka[:])
    nc.vector.tensor_copy(invk16b[:], invkb[:])
    with tc.tile_pool(name="psB", bufs=1, space="PSUM") as psB:
        rowp = psB.tile([1, 128], F16, tag="rp", name="rowp")
        nc.tensor.transpose(rowp[:], invk16a[:], id16[:])
        nc.vector.tensor_copy(invkrow[0:1, 0:128], rowp[:])
        rowp2 = psB.tile([1, 128], F16, tag="rp2", name="rowp2")
        nc.tensor.transpose(rowp2[0:1, 0:64], invk16b[:],
                            id16[0:64, 0:64])
        nc.vector.tensor_copy(invkrow[0:1, 128:C], rowp2[0:1, 0:64])
        ikA = psB.tile([128, C], F32, tag="ikA", name="ikA")
        nc.tensor.matmul(ikA[:], ones1[0:1, 0:128], invkrow[:],
                         start=True, stop=True)
        ikB = psB.tile([64, C], F32, tag="ikB", name="ikB")
        nc.tensor.matmul(ikB[:], ones1[0:1, 0:64], invkrow[:],
                         start=True, stop=True)
        nc.vector.tensor_copy(ikba[:], ikA[:])
        nc.vector.tensor_copy(ikbb[:], ikB[:])

    # ================= logits, softmax, M^T = A^T Wp^T =================
    with tc.tile_pool(name="smax", bufs=1) as sm, \
         tc.tile_pool(name="psM", bufs=1, space="PSUM") as psM:
        for Gx, ikx, spx, mkx, atx, rows in (
                (G0, ikba, spa, maska, at16a, 128),
                (G1, ikbb, spb, maskb, at16b, 64)):
            lg = sm.tile([rows, C], F32, tag=f"lg{rows}", name=f"lg{rows}")
            nc.vector.tensor_tensor(out=lg[:], in0=Gx[:], in1=ikx[:],
                                    op=mybir.AluOpType.mult)
            lg2 = sm.tile([rows, C], F32, tag=f"lh{rows}", name=f"lh{rows}")
            nc.vector.scalar_tensor_tensor(
                out=lg2[:], in0=lg[:], scalar=spx[:], in1=mkx[:],
                op0=mybir.AluOpType.mult, op1=mybir.AluOpType.add)
            mx = sm.tile([rows, 1], F32, tag=f"mx{rows}", name=f"mx{rows}")
            nc.vector.reduce_max(mx[:], lg2[:], axis=mybir.AxisListType.X)
            nc.vector.tensor_scalar_mul(mx[:], mx[:], -1.0)
            ssum = sm.tile([rows, 1], F32, tag=f"ss{rows}", name=f"ss{rows}")
            nc.scalar.activation(lg2[:], lg2[:],
                                 mybir.ActivationFunctionType.Exp,
                                 bias=mx[:], accum_out=ssum[:])
            nc.vector.reciprocal(ssum[:], ssum[:])
            nc.vector.tensor_scalar_mul(atx[:], lg2[:], ssum[:])
        # MT = A^T @ Wp^T  ([d, o], d on partitions)
        mta = psM.tile([128, C], F32, tag="mta", name="mta")
        nc.tensor.matmul(mta[:], at16a[:, 0:128], wpta[:],
                         start=True, stop=False)
        nc.tensor.matmul(mta[:], at16b[:, 0:128], wptb[:],
                         start=False, stop=True)
        mtb = psM.tile([64, C], F32, tag="mtb", name="mtb")
        nc.tensor.matmul(mtb[:], at16a[:, 128:C], wpta[:],
                         start=True, stop=False)
        nc.tensor.matmul(mtb[:], at16b[:, 128:C], wptb[:],
                         start=False, stop=True)
        nc.scalar.copy(mt16a[:], mta[:])
        nc.scalar.copy(mt16b[:], mtb[:])

    if DBG:
        gsb = sml.tile([128, C], F32)
        gsb2 = sml.tile([64, C], F32)
        nc.vector.tensor_copy(gsb[:], G0[:])
        nc.vector.tensor_copy(gsb2[:], G1[:])
        nc.sync.dma_start(io["dbg_ga"][0:128, :], gsb[:])
        nc.sync.dma_start(io["dbg_gb"][0:64, :], gsb2[:])
        nc.sync.dma_start(io["dbg_at"][0:128, :], at16a[:])
        nc.sync.dma_start(io["dbg_at"][128:C, :], at16b[:])
        nc.sync.dma_start(io["dbg_mt"][0:128, :], mt16a[:])
        nc.sync.dma_start(io["dbg_mt"][128:C, :], mt16b[:])
        nc.sync.dma_start(io["dbg_sp"][0:128, :], spa[:])
        nc.sync.dma_start(io["dbg_sp"][128:C, :], spb[:])
        nc.sync.dma_start(io["dbg_ik"][0:128, :], invka[:])
        nc.sync.dma_start(io["dbg_ik"][128:C, :], invkb[:])
        nc.sync.dma_start(io["dbg_k16"][0:128, :], k16a[:])
        nc.sync.dma_start(io["dbg_k16"][128:C, :], k16b[:])
        nc.sync.dma_start(io["dbg_v16"][0:128, :], va_all[:])
        nc.sync.dma_start(io["dbg_v16"][128:C, :], vb_all[:])
        nc.sync.dma_start(io["dbg_kba"][:, :], dbg_kba_t[:])
        nc.sync.dma_start(io["dbg_qba"][:, :], dbg_qba_t[:])

    # ================= out = M @ v =================
    with tc.tile_pool(name="ost", bufs=2) as ost, \
         tc.tile_pool(name="psO", bufs=1, space="PSUM") as psO:
        oa = ob = None
        for g0 in range(0, NT, 3):
            gts = list(range(g0, min(g0 + 3, NT)))
            O0s = {j: psO.tile([128, PT], F32, tag=f"O0{j - g0}", name="O0")
                   for j in gts}
            O1s = {j: psO.tile([64, PT], F32, tag=f"O1{j - g0}", name="O1")
                   for j in gts}
            for j in gts:
                nc.tensor.matmul(O0s[j][:], mt16a[:, 0:128], va_all[:, sl(j)],
                                 start=True, stop=False)
            for j in gts:
                nc.tensor.matmul(O0s[j][:], mt16b[:, 0:128], vb_all[:, sl(j)],
                                 start=False, stop=True)
            for j in gts:
                nc.tensor.matmul(O1s[j][:], mt16a[:, 128:C], va_all[:, sl(j)],
                                 start=True, stop=False)
            for j in gts:
                nc.tensor.matmul(O1s[j][:], mt16b[:, 128:C], vb_all[:, sl(j)],
                                 start=False, stop=True)
            for j in gts:
                c, jj = j // 4, j % 4
                if jj == 0:
                    oa = ost.tile([128, CHW], F32, tag="oa", name="oa")
                    ob = ost.tile([64, CHW], F32, tag="ob", name="ob")
                    oab = {}
                oab[c] = (oa, ob)
                nc.any.tensor_copy(oa[:, sl(jj)], O0s[j][:])
                nc.any.tensor_copy(ob[:, sl(jj)], O1s[j][:])
                if jj == 3:
                    nc.sync.dma_start(out[0:128, sl(c, CHW)], oa[:])
                    nc.sync.dma_start(out[128:C, sl(c, CHW)], ob[:])
    _stack.close()


def build_module():
    nc = bacc.Bacc("TRN2")
    io = {}
    io["kv"] = nc.dram_tensor("kv", [C, HWTOT], F32, kind="ExternalInput").ap()
    io["q"] = nc.dram_tensor("q", [C, HWTOT], F32, kind="ExternalInput").ap()
    io["w1t"] = nc.dram_tensor("w1t", [C, C2], F16, kind="ExternalInput").ap()
    io["w2d"] = nc.dram_tensor("w2d", [27, 128, 128], F16,
                               kind="ExternalInput").ap()
    io["wpt"] = nc.dram_tensor("wpt", [C, C], F16, kind="ExternalInput").ap()
    io["ident"] = nc.dram_tensor("ident", [128, 128], F16,
                                 kind="ExternalInput").ap()
    io["mask"] = nc.dram_tensor("mask", [C, C], F32, kind="ExternalInput").ap()
    io["scale192"] = nc.dram_tensor("scale192", [C, 1], F32,
                                    kind="ExternalInput").ap()
    io["out"] = nc.dram_tensor("out", [C, HWTOT], F32, kind="ExternalOutput").ap()
    io["vdram"] = nc.dram_tensor("vdram", [C, HWTOT], F16).ap()
    if DBG:
        io["dbg_ga"] = nc.dram_tensor("dbg_ga", [128, C], F32, kind="ExternalOutput").ap()
        io["dbg_gb"] = nc.dram_tensor("dbg_gb", [64, C], F32, kind="ExternalOutput").ap()
        io["dbg_at"] = nc.dram_tensor("dbg_at", [C, C], F16, kind="ExternalOutput").ap()
        io["dbg_mt"] = nc.dram_tensor("dbg_mt", [C, C], F16, kind="ExternalOutput").ap()
        io["dbg_sp"] = nc.dram_tensor("dbg_sp", [C, 1], F32, kind="ExternalOutput").ap()
        io["dbg_ik"] = nc.dram_tensor("dbg_ik", [C, 1], F32, kind="ExternalOutput").ap()
        io["dbg_k16"] = nc.dram_tensor("dbg_k16", [C, HWTOT], F16, kind="ExternalOutput").ap()
        io["dbg_v16"] = nc.dram_tensor("dbg_v16", [C, HWTOT], F16, kind="ExternalOutput").ap()
        io["dbg_kba"] = nc.dram_tensor("dbg_kba", [128, 4 * 128], F16, kind="ExternalOutput").ap()
        io["dbg_qba"] = nc.dram_tensor("dbg_qba", [128, 4 * 128], F16, kind="ExternalOutput").ap()
    with tile.TileContext(nc) as tc:
        emit_kernel(tc, io)
    nc.compile()
    return nc


def prep_weights(qkv1_w, qkv2_w, proj_w, scale):
    w1 = np.asarray(qkv1_w).reshape(C2, C)
    w1t = np.ascontiguousarray(w1.T).astype(np.float16)
    w2 = np.asarray(qkv2_w).reshape(C2, 9)
    w2d = np.zeros((27, 128, 128), np.float16)
    for mc in range(3):
        for wi in range(9):
            np.fill_diagonal(w2d[mc * 9 + wi], w2[mc * 128:(mc + 1) * 128, wi])
    wpr = np.asarray(proj_w).reshape(C, C)
    wpt = np.ascontiguousarray(wpr.T).astype(np.float16)
    ident = np.eye(128, dtype=np.float16)
    mask = np.full((C, C), -1e30, np.float32)
    for h in range(HEADS):
        mask[h * CD:(h + 1) * CD, h * CD:(h + 1) * CD] = 0.0
    scale192 = np.repeat(np.asarray(scale).reshape(HEADS), CD).astype(
        np.float32).reshape(C, 1)
    return {"w1t": w1t, "w2d": w2d, "wpt": wpt, "ident": ident,
            "mask": mask, "scale192": scale192}


_CACHED = {}


def kernel(kv, q, qkv1_w, qkv2_w, proj_w, scale):
    kv = np.asarray(kv, np.float32)
    q = np.asarray(q, np.float32)
    b = kv.shape[0]
    assert b == 8 and kv.shape[1] == C
    wts = prep_weights(qkv1_w, qkv2_w, proj_w, scale)
    if "nc" not in _CACHED:
        ncm = build_module()
        ncm.m = get_hw_module(ncm.m)
        _CACHED["nc"] = ncm
    ncm = _CACHED["nc"]
    in_maps = []
    for i in range(b):
        m = {"kv": np.ascontiguousarray(kv[i].reshape(C, HWTOT)),
             "q": np.ascontiguousarray(q[i].reshape(C, HWTOT))}
        m.update(wts)
        in_maps.append(m)
    res = run_bass_kernel_spmd(ncm, in_maps, core_ids=list(range(8)))
    outv = np.stack([res.results[i]["out"].reshape(C, H, W) for i in range(b)])
    return outv.astype(np.float32)
